# revision 10
# baseline (speedup 1.0000x reference)
"""GraphConv (DGL norm='both' + ELU) Trainium2 kernel, 8-way SPMD.

  out = ELU( Din^{-1/2} * A * Dout^{-1/2} * h @ W + b )

Strategy (dst-node sharding, graph preprocessing on host):
  - Nodes are packed into 128-node "blocks" (serpentine over in-degree order
    for edge balance); 8 cores x 98 blocks; h replicated per core as bf16.
  - The per-edge source-row gather is issued with the rows VIEWED AS int64
    (64 x i64 = 512B = 256 x bf16): the SWDGE gather is a byte mover, and the
    cost model charges per gathered element, so the i64 view quarters the
    Pool-engine time of the dominant gather term.
  - Edges are grouped by (dst block, 25000-row src window) [int16 idx limit];
    each group is padded to a 128-slot chunk multiple with a static chunk
    count c(b,w) = ceil(max-over-cores count / 128) so the SPMD program is
    identical across cores.  Gather calls cover a (superblock of 2 dst
    blocks) x (window) slot range (<=2048 idxs, SWDGE ring raised via
    dynamic_dma_scratch_size=32768).  Pad slots duplicate the last real idx
    (finite data) and are killed in S by dstcol=999.
  - Weighted segment-sum on the TensorEngine: per 128-edge chunk a selection
    matrix S[p,d] = (iota==dstcol[p])*coef[p] (coef = Dout^{-1/2}[src], bf16
    out, one DVE op at 2x 16-bit rate) is matmul'd with the chunk rows into a
    PSUM accumulator agg[128 dst, 256].
  - Din^{-1/2} is applied as a per-partition ACT scale on the PSUM->SBUF copy
    (bf16 out), agg is transposed via PE (bf16, 1 cyc/row), multiplied by W
    (bf16, 1 cyc/row vs 4 for f32) with bias folded in as a K=1 ones x bias
    matmul, and ELU'd (relu/exp on ACT + one fused DVE op).
  - Host un-permutes the 8 core outputs back to node order.
"""

import os
import sys

import numpy as np

try:
    import concourse.bass as bass
except ImportError:  # fresh grading dir: concourse comes from the container env
    for _p in ("/opt/trn_rl_repo", "/root/.axon_site/_ro/trn_rl_repo"):
        if os.path.isdir(_p) and _p not in sys.path:
            sys.path.append(_p)
    import concourse.bass as bass

import time

import ml_dtypes
import concourse.tile as tile
from concourse import bacc, mybir

# ---------------------------------------------------------------------------
# Problem config (hardcoded per the task statement)
# ---------------------------------------------------------------------------
N_NODES = 100000
DIM = 256
CORES = 8
P = 128
WIN = 25000  # src gather window (int16 indices must stay < 32768)
N_WIN = (N_NODES + WIN - 1) // WIN  # 4
BPC = (N_NODES + P * CORES - 1) // (P * CORES)  # 98 blocks per core
SB = 4  # dst blocks per gather superblock
PACK = 8  # bytes per gathered element (int64 view)
ELEM = DIM * 2 // PACK  # 64 i64 elements per 512B bf16 row
MAX_CALL = 1024  # SWDGE descriptor-ring limit per gather call (hard hw cap)


def _sb_list():
    """Superblock partition of the BPC blocks: [6,6,...,6,2]."""
    out = []
    b = 0
    while b < BPC:
        n = min(SB, BPC - b)
        out.append(list(range(b, b + n)))
        b += n
    return out

F32 = mybir.dt.float32
BF16 = mybir.dt.bfloat16
I16 = mybir.dt.int16
I64 = mybir.dt.int64

BF16_NP = ml_dtypes.bfloat16

STAGE = int(os.environ.get("K_STAGE", "3"))  # 1=gather only, 2=+segsum, 3=full


class _Plan:
    """Host-side graph partitioning + per-core device input arrays."""

    def __init__(self, h, weight, bias, src, dst):
        n = h.shape[0]
        assert n == N_NODES and h.shape[1] == DIM
        e = src.shape[0]
        nb = BPC * CORES

        deg_out = np.bincount(src, minlength=n).astype(np.float32)
        deg_in = np.bincount(dst, minlength=n).astype(np.float32)
        a_src = 1.0 / np.sqrt(np.maximum(deg_out, 1.0))
        b_dst = 1.0 / np.sqrt(np.maximum(deg_in, 1.0))

        # --- node -> (block, pos): serpentine over in-degree order ---
        order = np.argsort(-deg_in, kind="stable")
        padded = np.concatenate([order, np.full(nb * P - n, -1, np.int64)])
        grid = padded.reshape(P, nb)
        grid[1::2] = grid[1::2, ::-1]
        self.grid = grid  # [P, nb]; grid[r, b] = node id or -1
        mask = grid >= 0
        node_block = np.empty(n, np.int64)
        node_pos = np.empty(n, np.int64)
        b_idx = np.broadcast_to(np.arange(nb), (P, nb))
        r_idx = np.broadcast_to(np.arange(P)[:, None], (P, nb))
        node_block[grid[mask]] = b_idx[mask]
        node_pos[grid[mask]] = r_idx[mask]

        # --- per-(core, block, window) counts -> static chunk table ---
        eb = node_block[dst]  # global block id; core = eb // BPC
        ecore = eb // BPC
        eblk = eb % BPC
        ew = src // WIN
        gkey = (ecore * BPC + eblk) * N_WIN + ew
        counts = np.bincount(gkey, minlength=CORES * BPC * N_WIN)
        counts = counts.reshape(CORES, BPC, N_WIN)
        maxc = counts.max(axis=0)  # [BPC, N_WIN]
        self.c_tab = -(-maxc // P)  # chunks per (block, window), static
        # group order: for sb: for w: for b in sb (window-major runs inside a
        # superblock so gather calls cover contiguous single-window slots)
        ords = []
        for blocks in _sb_list():
            for w in range(N_WIN):
                for b in blocks:
                    ords.append((b, w))
        self.group_order = ords
        gidx_of = {bw: i for i, bw in enumerate(ords)}
        c_seq = np.array([self.c_tab[b, w] for (b, w) in ords], np.int64)
        chunk_base = np.zeros(len(ords) + 1, np.int64)
        np.cumsum(c_seq, out=chunk_base[1:])
        self.c_seq = c_seq
        self.chunk_base = chunk_base  # chunk index base per ordered group
        self.total_chunks = int(chunk_base[-1])
        self.total_slots = self.total_chunks * P

        # --- per-core slot fill (vectorized) ---
        # order edges by (core, ordered-group, arbitrary)
        g_ord = np.empty(BPC * N_WIN, np.int64)  # (b, w) -> order pos
        for i, (b, w) in enumerate(ords):
            g_ord[b * N_WIN + w] = i
        e_ord = ecore * len(ords) + g_ord[eblk * N_WIN + ew]
        perm = np.argsort(e_ord, kind="stable")
        s_src = src[perm]
        s_dst = dst[perm]
        s_ord = e_ord[perm]
        grp_counts = np.bincount(e_ord, minlength=CORES * len(ords))
        grp_starts = np.zeros(CORES * len(ords) + 1, np.int64)
        np.cumsum(grp_counts, out=grp_starts[1:])
        within = np.arange(e) - grp_starts[s_ord]

        slot_base = np.tile(chunk_base[:-1] * P, CORES) + (
            np.repeat(np.arange(CORES), len(ords)) * self.total_slots
        )
        e_slot = slot_base[s_ord] + within  # global slot id over all cores

        ts = self.total_slots
        idx_flat = np.zeros(CORES * ts, np.int16)
        dstcol = np.full(CORES * ts, 999.0, np.float32)
        coef = np.zeros(CORES * ts, np.float32)
        idx_flat[e_slot] = (s_src % WIN).astype(np.int16)
        dstcol[e_slot] = node_pos[s_dst].astype(np.float32)
        coef[e_slot] = a_src[s_src]
        # pad slots keep idx 0 (row 0 of the call's window: always a valid,
        # finite h row) and are killed in S by dstcol=999 / coef=0.

        # --- idx SBUF layout: [128, total_slots/16], 16-wrap + 8x replicate ---
        L = idx_flat.reshape(CORES, ts // 16, 16)
        idx_sb = np.ascontiguousarray(
            np.broadcast_to(
                L.transpose(0, 2, 1)[:, None, :, :], (CORES, 8, 16, ts // 16)
            ).reshape(CORES, P, ts // 16)
        )
        self.idx_sb = idx_sb

        # --- per-chunk scalar arrays [128, total_chunks] ---
        self.dstcol_sb = np.ascontiguousarray(
            dstcol.reshape(CORES, self.total_chunks, P).transpose(0, 2, 1)
        )
        self.coef_sb = np.ascontiguousarray(
            coef.reshape(CORES, self.total_chunks, P).transpose(0, 2, 1)
        )

        bd = np.ones((P, nb), np.float32)
        bd[mask] = b_dst[grid[mask]]
        self.bdst_sb = np.ascontiguousarray(
            bd.reshape(P, CORES, BPC).transpose(1, 0, 2)
        )
        self.iota = np.ascontiguousarray(
            np.broadcast_to(np.arange(P, dtype=np.float32), (P, P))
        ).astype(BF16_NP)
        self.ident = np.eye(P, dtype=np.float32).astype(BF16_NP)
        self.weight = np.ascontiguousarray(weight, np.float32).astype(BF16_NP)
        self.bias = (
            np.ascontiguousarray(bias, np.float32).astype(BF16_NP).reshape(1, DIM)
        )
        h_bf16 = np.ascontiguousarray(h, np.float32).astype(BF16_NP)
        # u32 view at the PJRT boundary (int64 inputs are rejected by the
        # neuron client); the program bitcasts to i64 for the gather.
        self.h_u32 = h_bf16.view(np.uint32)  # [N, 128]

    def in_maps(self):
        maps = []
        for k in range(CORES):
            maps.append(
                {
                    "h": self.h_u32,
                    "weight": self.weight,
                    "bias": self.bias,
                    "iota": self.iota,
                    "ident": self.ident,
                    "idx": self.idx_sb[k],
                    "dstcol": self.dstcol_sb[k],
                    "coef": self.coef_sb[k],
                    "bdst": self.bdst_sb[k],
                }
            )
        return maps

    def assemble(self, results):
        out = np.empty((N_NODES, DIM), np.float32)
        for k in range(CORES):
            rows = results[k]["out"].reshape(BPC, P, DIM)
            g = self.grid[:, k * BPC : (k + 1) * BPC]  # [P, BPC]
            m = g >= 0
            out[g.T[m.T]] = rows[m.T]
        return out


def _build_program(plan):
    """Trace the SPMD Tile program (identical across cores)."""
    nc = bacc.Bacc(
        "TRN2",
        target_bir_lowering=False,
        debug=False,
        num_devices=CORES,
        num_swdge_queues=4,
    )
    c_tab = plan.c_tab  # [BPC, N_WIN] chunks per group
    chunk_base = plan.chunk_base
    ords = plan.group_order
    gidx = {bw: i for i, bw in enumerate(ords)}
    TC = plan.total_chunks
    TS = plan.total_slots
    sb_blocks = _sb_list()
    # max chunks within one superblock (for the ebuf tile size)
    sb_chunks = [
        int(sum(c_tab[b, w] for w in range(N_WIN) for b in blocks))
        for blocks in sb_blocks
    ]
    CMAX = max(sb_chunks)

    h = nc.dram_tensor("h", [N_NODES, DIM // 2], mybir.dt.uint32, kind="ExternalInput").ap()
    weight = nc.dram_tensor("weight", [DIM, DIM], BF16, kind="ExternalInput").ap()
    biasrow = nc.dram_tensor("bias", [1, DIM], BF16, kind="ExternalInput").ap()
    iota_d = nc.dram_tensor("iota", [P, P], BF16, kind="ExternalInput").ap()
    ident_d = nc.dram_tensor("ident", [P, P], BF16, kind="ExternalInput").ap()
    idx_d = nc.dram_tensor("idx", [P, TS // 16], I16, kind="ExternalInput").ap()
    dstcol_d = nc.dram_tensor("dstcol", [P, TC], F32, kind="ExternalInput").ap()
    coef_d = nc.dram_tensor("coef", [P, TC], F32, kind="ExternalInput").ap()
    bdst_d = nc.dram_tensor("bdst", [P, BPC], F32, kind="ExternalInput").ap()
    out_d = nc.dram_tensor("out", [BPC * P, DIM], F32, kind="ExternalOutput").ap()

    with tile.TileContext(nc) as tc:
        with (
            tc.tile_pool(name="resident", bufs=1) as res,
            tc.tile_pool(name="edges", bufs=3) as epool,
            tc.tile_pool(name="work", bufs=3) as wpool,
            tc.tile_pool(name="spool", bufs=4) as spool,
            tc.tile_pool(name="psum", bufs=2, space="PSUM") as ppool,
        ):
            iota_t = res.tile([P, P], BF16)
            nc.sync.dma_start(iota_t[:], iota_d[:])
            ident = res.tile([P, P], BF16)
            nc.sync.dma_start(ident[:], ident_d[:])
            w_t = res.tile([P, 2, DIM], BF16)
            nc.sync.dma_start(w_t[:, 0, :], weight[0:P, :])
            nc.sync.dma_start(w_t[:, 1, :], weight[P:DIM, :])
            bias_t = res.tile([1, DIM], BF16)
            nc.sync.dma_start(bias_t[:], biasrow[:])
            ones_t = res.tile([1, P], BF16)
            nc.vector.memset(ones_t[:], 1.0)
            idx_t = res.tile([P, TS // 16], I16)
            nc.sync.dma_start(idx_t[:], idx_d[:])
            dstcol_t = res.tile([P, TC], F32)
            nc.sync.dma_start(dstcol_t[:], dstcol_d[:])
            coef_t = res.tile([P, TC], F32)
            nc.sync.dma_start(coef_t[:], coef_d[:])
            bdst_t = res.tile([P, BPC], F32)
            nc.sync.dma_start(bdst_t[:], bdst_d[:])

            qrot = 0
            for sbi, blocks in enumerate(sb_blocks):
                ebuf = epool.tile([P, CMAX, ELEM], I64, tag="ebuf")
                # gather calls: per window, the superblock's groups form one
                # contiguous slot run; split into <=1024-idx calls (hw ring).
                cb0 = chunk_base[gidx[(blocks[0], 0)]]  # first chunk of this sb
                for w in range(N_WIN):
                    g0 = gidx[(blocks[0], w)]
                    run = int(sum(c_tab[b, w] for b in blocks))
                    lo = w * WIN
                    hi = min(lo + WIN, N_NODES)
                    done = 0
                    while done < run:
                        ncall = min(run - done, MAX_CALL // P)
                        gc0 = chunk_base[g0] + done  # global chunk id
                        c0 = gc0 - cb0  # ebuf chunk offset
                        nc.gpsimd.dma_gather(
                            ebuf[:, c0 : c0 + ncall, :],
                            h[lo:hi, :].bitcast(I64),
                            idx_t[:, gc0 * 8 : (gc0 + ncall) * 8],
                            ncall * P,
                            ncall * P,
                            ELEM,
                            queue_num=qrot % 4,
                        )
                        qrot += 1
                        done += ncall

                for b in blocks:
                    if STAGE == 1:
                        o_t = wpool.tile([P, DIM], F32, tag="out")
                        g0 = gidx[(b, 0)]
                        c0 = chunk_base[g0] - cb0
                        nc.vector.tensor_copy(
                            o_t[:], ebuf[:, c0, :].bitcast(F32)
                        )
                        nc.sync.dma_start(out_d[b * P : (b + 1) * P, :], o_t[:])
                        continue
                    # --- weighted segment-sum via PE ---
                    agg_ps = ppool.tile([P, DIM], F32, tag="agg")
                    bchunks = []
                    for w in range(N_WIN):
                        g0 = gidx[(b, w)]
                        for c in range(int(c_tab[b, w])):
                            bchunks.append((chunk_base[g0] + c, chunk_base[g0] + c - cb0))
                    for ci, (gc, ec) in enumerate(bchunks):
                        s_t = spool.tile([P, P], BF16, tag="sel")
                        nc.vector.tensor_scalar(
                            s_t[:],
                            iota_t[:],
                            dstcol_t[:, gc : gc + 1],
                            coef_t[:, gc : gc + 1],
                            mybir.AluOpType.is_equal,
                            mybir.AluOpType.mult,
                        )
                        nc.tensor.matmul(
                            agg_ps[:],
                            lhsT=s_t[:],
                            rhs=ebuf[:, ec, :].bitcast(BF16),
                            start=(ci == 0),
                            stop=(ci == len(bchunks) - 1),
                        )

                    # --- scale by Din^{-1/2}, transpose, @W + bias, ELU ---
                    agg_sb = wpool.tile([P, DIM], BF16, tag="aggsb")
                    nc.scalar.activation(
                        agg_sb[:],
                        agg_ps[:],
                        mybir.ActivationFunctionType.Copy,
                        scale=bdst_t[:, b : b + 1],
                    )
                    if STAGE == 2:
                        o_t = wpool.tile([P, DIM], F32, tag="out")
                        nc.vector.tensor_copy(o_t[:], agg_sb[:])
                        nc.sync.dma_start(out_d[b * P : (b + 1) * P, :], o_t[:])
                        continue
                    aggT_ps = ppool.tile([P, DIM], BF16, tag="aggT")
                    nc.tensor.transpose(aggT_ps[:, 0:P], agg_sb[:, 0:P], ident[:])
                    nc.tensor.transpose(aggT_ps[:, P:DIM], agg_sb[:, P:DIM], ident[:])
                    aggT_sb = wpool.tile([P, DIM], BF16, tag="aggTsb")
                    nc.scalar.activation(
                        aggT_sb[:], aggT_ps[:], mybir.ActivationFunctionType.Copy
                    )

                    z_ps = ppool.tile([P, DIM], F32, tag="z")
                    nc.tensor.matmul(
                        z_ps[:], lhsT=ones_t[:], rhs=bias_t[:], start=True, stop=False
                    )
                    nc.tensor.matmul(
                        z_ps[:],
                        lhsT=aggT_sb[:, 0:P],
                        rhs=w_t[:, 0, :],
                        start=False,
                        stop=False,
                    )
                    nc.tensor.matmul(
                        z_ps[:],
                        lhsT=aggT_sb[:, P:DIM],
                        rhs=w_t[:, 1, :],
                        start=False,
                        stop=True,
                    )

                    # ELU(z) = relu(z) + exp(-relu(-z)) - 1
                    r_t = wpool.tile([P, DIM], F32, tag="relu")
                    nc.scalar.activation(
                        r_t[:], z_ps[:], mybir.ActivationFunctionType.Relu
                    )
                    rn_t = wpool.tile([P, DIM], F32, tag="rneg")
                    nc.scalar.activation(
                        rn_t[:], z_ps[:], mybir.ActivationFunctionType.Relu, scale=-1.0
                    )
                    e_t = wpool.tile([P, DIM], F32, tag="exp")
                    nc.scalar.activation(
                        e_t[:], rn_t[:], mybir.ActivationFunctionType.Exp, scale=-1.0
                    )
                    o_t = wpool.tile([P, DIM], F32, tag="out")
                    nc.vector.scalar_tensor_tensor(
                        o_t[:],
                        r_t[:],
                        -1.0,
                        e_t[:],
                        mybir.AluOpType.add,
                        mybir.AluOpType.add,
                    )
                    nc.sync.dma_start(out_d[b * P : (b + 1) * P, :], o_t[:])

    nc.compile()
    return nc


# ---------------------------------------------------------------------------
# Execution via PJRT on the axon-tunneled NeuronCores (adapted from
# concourse.bass2jax.run_bass_via_pjrt, pinned to the axon/neuron platform).
# ---------------------------------------------------------------------------
_EXEC_CACHE = {}


def _axon_devices():
    import jax

    try:
        return jax.devices("axon")
    except RuntimeError:
        return jax.devices()


def _make_executor(nc):
    import jax
    import numpy as _np
    from jax.sharding import Mesh, PartitionSpec
    from jax.experimental.shard_map import shard_map
    from concourse import bass2jax
    from concourse import mybir as mb

    bass2jax.install_neuronx_cc_hook()
    partition_name = nc.partition_id_tensor.name if nc.partition_id_tensor else None

    in_names, out_names, out_avals, zero_outs = [], [], [], []
    for alloc in nc.m.functions[0].allocations:
        if not isinstance(alloc, mb.MemoryLocationSet):
            continue
        name = alloc.memorylocations[0].name
        if alloc.kind == "ExternalInput":
            if name != partition_name:
                in_names.append(name)
        elif alloc.kind == "ExternalOutput":
            out_names.append(name)
            shape = tuple(alloc.tensor_shape)
            dtype = mb.dt.np(alloc.dtype)
            out_avals.append(jax.core.ShapedArray(shape, dtype))
            zero_outs.append(_np.zeros(shape, dtype))
    n_params = len(in_names)
    n_outs = len(out_avals)
    all_names = in_names + out_names + ([partition_name] if partition_name else [])

    def _body(*args):
        operands = list(args)
        if partition_name is not None:
            operands.append(bass2jax.partition_id_tensor())
        outs = bass2jax._bass_exec_p.bind(
            *operands,
            out_avals=tuple(out_avals),
            in_names=tuple(all_names),
            out_names=tuple(out_names),
            lowering_input_output_aliases=(),
            sim_require_finite=True,
            sim_require_nnan=True,
            nc=nc,
        )
        return tuple(outs)

    devices = _axon_devices()[:CORES]
    assert len(devices) == CORES, f"need {CORES} axon devices, got {len(devices)}"
    mesh = Mesh(np.asarray(devices), ("core",))
    in_specs = (PartitionSpec("core"),) * (n_params + n_outs)
    out_specs = (PartitionSpec("core"),) * n_outs
    fn = jax.jit(
        shard_map(
            _body, mesh=mesh, in_specs=in_specs, out_specs=out_specs, check_rep=False
        ),
        keep_unused=True,
    )
    return fn, in_names, out_names, zero_outs, mesh


def _execute(nc, in_maps, time_iters=0):
    key = id(nc)
    if key not in _EXEC_CACHE:
        _EXEC_CACHE.clear()
        _EXEC_CACHE[key] = _make_executor(nc)
    fn, in_names, out_names, zero_outs, mesh = _EXEC_CACHE[key]

    concat_in = [
        np.concatenate([np.asarray(in_maps[c][n]) for c in range(CORES)], axis=0)
        for n in in_names
    ]
    concat_zero = [np.concatenate([z for _ in range(CORES)], axis=0) for z in zero_outs]
    args = concat_in + concat_zero
    outs = fn(*args)
    outs = [np.asarray(o) for o in outs]

    exec_ns = None
    if time_iters:
        import jax
        from jax.sharding import NamedSharding, PartitionSpec

        shard = NamedSharding(mesh, PartitionSpec("core"))
        dargs = [jax.device_put(a, shard) for a in args]
        jax.block_until_ready(fn(*dargs))
        times = []
        for _ in range(time_iters):
            t0 = time.perf_counter()
            r = fn(*dargs)
            jax.block_until_ready(r)
            times.append(time.perf_counter() - t0)
        exec_ns = min(times) * 1e9

    results = []
    for c in range(CORES):
        m = {}
        for i, nme in enumerate(out_names):
            per = outs[i].shape[0] // CORES
            m[nme] = outs[i][c * per : (c + 1) * per]
        results.append(m)
    return results, exec_ns


_PROGRAM_CACHE = {}


def _get_plan_and_program(h, weight, bias, src, dst):
    plan = _Plan(h, weight, bias, src, dst)
    pkey = (plan.total_chunks, tuple(plan.c_seq.tolist()))
    if pkey not in _PROGRAM_CACHE:
        _PROGRAM_CACHE.clear()
        _PROGRAM_CACHE[pkey] = _build_program(plan)
    return plan, _PROGRAM_CACHE[pkey]


def kernel(h, weight, bias, src, dst, _time_iters=0):
    h = np.asarray(h, np.float32)
    weight = np.asarray(weight, np.float32)
    bias = np.asarray(bias, np.float32)
    src = np.asarray(src, np.int32)
    dst = np.asarray(dst, np.int32)
    plan, nc = _get_plan_and_program(h, weight, bias, src, dst)
    results, exec_ns = _execute(nc, plan.in_maps(), time_iters=_time_iters)
    out = plan.assemble(results)
    if _time_iters:
        kernel.last_exec_ns = exec_ns
    return out
